# revision 1
# baseline (speedup 1.0000x reference)
"""Causal GQA self-attention (B=2, T=2048, D=2048, 16 q-heads / 4 kv-heads,
head_dim=128, full-dim RoPE) on 8 Trainium2 NeuronCores.

Strategy: tensor-parallel over heads. Core c owns q-heads {2c, 2c+1} and
kv-head c//2. Wq/Wkv output dims and Wproj input dims are sharded 8-ways on
the host; each core computes a full-width partial of the output projection
and the host sums the 8 partials.

On-chip layout: x is staged transposed (channel-major [C, B*T]) so the
QKV projections need no on-chip transpose; attention is computed "k-major"
(scores transposed, [k_pos, q_pos]) so the P@V contraction needs no
transpose either. Softmax runs without max-subtraction (scores are ~N(0,1);
exp never overflows) and the denominator comes from an all-ones stationary
matmul which also broadcasts the per-column sums across all partitions.
"""

import math
import os
import sys

for _p in ("/opt/trn_rl_repo", "/root/.axon_site/_ro/trn_rl_repo"):
    if os.path.isdir(_p) and _p not in sys.path:
        sys.path.insert(0, _p)

import ml_dtypes
import numpy as np

BF16 = ml_dtypes.bfloat16

B = 2
T = 2048
C = 2048
D = 128          # head dim
NQH = 2          # q heads per core
TOK = B * T      # 4096
KT = C // 128    # 16 contraction tiles
NCH = 512        # matmul moving-dim chunk
QCH = T // NCH   # 4 q chunks per batch
KB = T // 128    # 16 k tiles per batch
N_CORES = 8
SCALE = 1.0 / math.sqrt(D)

_COMPILED = {}


def _rope_tables():
    dim = np.arange(D // 2, dtype=np.float64)
    freq = 10000.0 ** (dim / (D / 2))
    freq = np.concatenate([freq, freq])              # [128]
    pos = np.arange(T, dtype=np.float64)
    ang = pos[None, :] / freq[:, None]               # [128, T] channel-major
    return np.cos(ang), np.sin(ang)


def _build_nc(debug=False):
    import concourse.bass as bass  # noqa: F401
    import concourse.mybir as mybir
    import concourse.tile as tile
    from concourse import bacc
    from concourse.bass import ts

    f32 = mybir.dt.float32
    bf16 = mybir.dt.bfloat16
    AF = mybir.ActivationFunctionType
    OP = mybir.AluOpType

    nc = bacc.Bacc("TRN2", target_bir_lowering=False, debug=False,
                   num_devices=N_CORES)

    xt_e = nc.dram_tensor("xt", [C, TOK], bf16, kind="ExternalInput")
    wq_e = nc.dram_tensor("wq", [C, NQH * D], bf16, kind="ExternalInput")
    wk_e = nc.dram_tensor("wk", [C, D], bf16, kind="ExternalInput")
    wv_e = nc.dram_tensor("wv", [C, D], bf16, kind="ExternalInput")
    wp_e = nc.dram_tensor("wp", [NQH * D, C], bf16, kind="ExternalInput")
    cos_e = nc.dram_tensor("cos", [D, T], bf16, kind="ExternalInput")
    sin_e = nc.dram_tensor("sin", [D, T], bf16, kind="ExternalInput")
    tri_e = nc.dram_tensor("tri", [D, D], bf16, kind="ExternalInput")
    out_e = nc.dram_tensor("out", [TOK, C], f32, kind="ExternalOutput")
    if debug:
        dbg_q = nc.dram_tensor("dbg_q", [D, NQH * TOK], bf16, kind="ExternalOutput")
        dbg_k = nc.dram_tensor("dbg_k", [D, TOK], bf16, kind="ExternalOutput")
        dbg_v = nc.dram_tensor("dbg_v", [128, B * KB * D], bf16, kind="ExternalOutput")
        dbg_y = nc.dram_tensor("dbg_y", [D, NQH * TOK], bf16, kind="ExternalOutput")

    from contextlib import ExitStack

    with tile.TileContext(nc) as tc, ExitStack() as ctx:
        const = ctx.enter_context(tc.tile_pool(name="const", bufs=1))
        qkvp = ctx.enter_context(tc.tile_pool(name="qkv", bufs=1))
        psum = ctx.enter_context(tc.tile_pool(name="ps", bufs=3, space="PSUM"))
        xtp = ctx.enter_context(tc.tile_pool(name="xt", bufs=1))
        w1p = ctx.enter_context(tc.tile_pool(name="w1", bufs=1))
        rtp = ctx.enter_context(tc.tile_pool(name="rt", bufs=3))
        exp_p = ctx.enter_context(tc.tile_pool(name="exp", bufs=6))
        recp = ctx.enter_context(tc.tile_pool(name="rec", bufs=2))
        outp = ctx.enter_context(tc.tile_pool(name="outs", bufs=3))

        # DMA emission order tuned so the first q-projection can start
        # after ~3MB: wq + the first 512-token chunk of xt come first.
        wq_sb = w1p.tile([128, KT, NQH * D], bf16, tag="wq")
        nc.sync.dma_start(wq_sb[:], wq_e.ap().rearrange("(ko p) n -> p ko n", p=128))
        xt0_sb = xtp.tile([128, KT, T], bf16, tag="xt")
        for kt in range(KT):
            nc.sync.dma_start(xt0_sb[:, kt, 0:NCH],
                              xt_e.ap()[kt * 128:(kt + 1) * 128, 0:NCH])
        cos_sb = const.tile([D, T], bf16, tag="cos")
        nc.sync.dma_start(cos_sb[:], cos_e.ap())
        sin_sb = const.tile([D, T], bf16, tag="sin")
        nc.sync.dma_start(sin_sb[:], sin_e.ap())
        wk_sb = w1p.tile([128, KT, D], bf16, tag="wk")
        nc.sync.dma_start(wk_sb[:], wk_e.ap().rearrange("(ko p) n -> p ko n", p=128))
        wv_sb = w1p.tile([128, KT, D], bf16, tag="wv")
        nc.sync.dma_start(wv_sb[:], wv_e.ap().rearrange("(ko p) n -> p ko n", p=128))
        tri_sb = const.tile([D, D], bf16, tag="tri")
        nc.sync.dma_start(tri_sb[:], tri_e.ap())
        ones_sb = const.tile([128, 128], bf16, tag="ones")
        nc.vector.memset(ones_sb[:], 1.0)
        wp_sb = const.tile([128, NQH, C], bf16, tag="wp")
        nc.sync.dma_start(wp_sb[:], wp_e.ap().rearrange("(ko p) n -> p ko n", p=128))

        # persistent per-batch-pair tensors
        qT = qkvp.tile([D, NQH, TOK], bf16, tag="qT")    # rope'd, pre-scaled
        kT = qkvp.tile([D, TOK], bf16, tag="kT")         # rope'd
        vv = qkvp.tile([128, B * KB, D], bf16, tag="vv")  # token-major
        yT = qkvp.tile([D, NQH, TOK], bf16, tag="yT")    # attn out, normalized

        def rope_out(dst, src_ps, cos_ap, sin_ap):
            """dst(bf16 sbuf) = src_ps * cos + rotate_half(src_ps) * sin."""
            rt = rtp.tile([128, NCH], f32, tag="rt")
            nc.vector.tensor_scalar(out=rt[0:64, :], in0=src_ps[64:128, :],
                                    scalar1=-1.0, scalar2=None, op0=OP.mult)
            nc.vector.tensor_copy(out=rt[64:128, :], in_=src_ps[0:64, :])
            m1 = rtp.tile([128, NCH], f32, tag="m1")
            nc.vector.tensor_tensor(out=m1[:], in0=src_ps[:], in1=cos_ap, op=OP.mult)
            m2 = rtp.tile([128, NCH], f32, tag="m2")
            nc.vector.tensor_tensor(out=m2[:], in0=rt[:], in1=sin_ap, op=OP.mult)
            nc.vector.tensor_tensor(out=dst, in0=m1[:], in1=m2[:], op=OP.add)

        def emit_proj(b, qc):
            tok0 = b * T
            for qt in range(4 * qc, 4 * qc + 4):
                osb = outp.tile([128, C], f32, tag="osb")
                for fc in range(C // NCH):
                    ops = psum.tile([128, NCH], f32, tag="mm")
                    for kd in range(NQH):
                        nc.tensor.matmul(
                            ops[:],
                            yT[:, kd, tok0 + qt * 128: tok0 + (qt + 1) * 128],
                            wp_sb[:, kd, ts(fc, NCH)],
                            start=(kd == 0), stop=(kd == NQH - 1))
                    if fc % 2 == 0:
                        nc.scalar.copy(osb[:, ts(fc, NCH)], ops[:])
                    else:
                        nc.vector.tensor_copy(osb[:, ts(fc, NCH)], ops[:])
                nc.sync.dma_start(
                    out_e.ap()[tok0 + qt * 128: tok0 + (qt + 1) * 128, :],
                    osb[:])

        pending = None
        for b in range(B):
            tok0 = b * T
            # ---- phase 1: QKV projection + RoPE for batch b ----
            # chunk-major loads so the first projections start after ~2MB
            if b == 0:
                xt_sb = xt0_sb  # chunk 0 DMAs already emitted up top
                for tc_ in range(1, QCH):
                    for kt in range(KT):
                        nc.sync.dma_start(
                            xt_sb[:, kt, ts(tc_, NCH)],
                            xt_e.ap()[kt * 128:(kt + 1) * 128,
                                      tok0 + tc_ * NCH: tok0 + (tc_ + 1) * NCH])
            else:
                xt_sb = xtp.tile([128, KT, T], bf16, tag="xt")
                for tc_ in range(QCH):
                    for kt in range(KT):
                        nc.sync.dma_start(
                            xt_sb[:, kt, ts(tc_, NCH)],
                            xt_e.ap()[kt * 128:(kt + 1) * 128,
                                      tok0 + tc_ * NCH: tok0 + (tc_ + 1) * NCH])
            for tc_ in range(QCH):
                for h in range(NQH):
                    ps = psum.tile([128, NCH], f32, tag="mm")
                    for kt in range(KT):
                        nc.tensor.matmul(ps[:],
                                         wq_sb[:, kt, h * D:(h + 1) * D],
                                         xt_sb[:, kt, ts(tc_, NCH)],
                                         start=(kt == 0), stop=(kt == KT - 1))
                    rope_out(qT[:, h, tok0 + tc_ * NCH: tok0 + (tc_ + 1) * NCH],
                             ps, cos_sb[:, ts(tc_, NCH)], sin_sb[:, ts(tc_, NCH)])
                ps = psum.tile([128, NCH], f32, tag="mm")
                for kt in range(KT):
                    nc.tensor.matmul(ps[:], wk_sb[:, kt, :],
                                     xt_sb[:, kt, ts(tc_, NCH)],
                                     start=(kt == 0), stop=(kt == KT - 1))
                rope_out(kT[:, tok0 + tc_ * NCH: tok0 + (tc_ + 1) * NCH],
                         ps, cos_sb[:, ts(tc_, NCH)], sin_sb[:, ts(tc_, NCH)])
                for ti in range(4 * tc_, 4 * tc_ + 4):
                    ps = psum.tile([128, D], f32, tag="mm")
                    for kt in range(KT):
                        nc.tensor.matmul(ps[:],
                                         xt_sb[:, kt, ti * 128:(ti + 1) * 128],
                                         wv_sb[:, kt, :],
                                         start=(kt == 0), stop=(kt == KT - 1))
                    nc.scalar.copy(vv[:, b * KB + ti, :], ps[:])

            # ---- phase 2+3: attention + out-projection for batch b ----
            # proj emission is delayed one chunk so the PE stream always has
            # the next attention chunk ahead of each proj (hides the
            # reciprocal->normalize chain on DVE).
            for qc in range(QCH):
                for h in range(NQH):
                    n_kt = 4 * qc + 4
                    yps = psum.tile([128, NCH], f32, tag="y", bufs=3)
                    sps = psum.tile([128, NCH], f32, tag="s", bufs=2)
                    for kti in range(n_kt):
                        dq = kti - 4 * qc
                        c0 = dq * 128 if dq > 0 else 0  # masked cols skipped
                        q_sl = qT[:, h, tok0 + qc * NCH + c0:
                                  tok0 + (qc + 1) * NCH]
                        sc = psum.tile([128, NCH], f32, tag="mm")
                        nc.tensor.matmul(sc[:, c0:],
                                         kT[:, tok0 + kti * 128: tok0 + (kti + 1) * 128],
                                         q_sl, start=True, stop=True)
                        ex = exp_p.tile([128, NCH], bf16, tag="ex")
                        nc.scalar.activation(ex[:, c0:], sc[:, c0:], AF.Exp)
                        if dq >= 0:
                            nc.vector.tensor_mul(ex[:, ts(dq, 128)],
                                                 ex[:, ts(dq, 128)], tri_sb[:])
                        st = (kti == 0)
                        sp = (kti == n_kt - 1)
                        nc.tensor.matmul(yps[:, c0:], vv[:, b * KB + kti, :],
                                         ex[:, c0:], start=st, stop=sp)
                        nc.tensor.matmul(sps[:, c0:], ones_sb[:], ex[:, c0:],
                                         start=st, stop=sp)
                    # free the PSUM accumulators quickly (ACT copy), then
                    # normalize in SBUF off the PE-critical path
                    ysb = recp.tile([128, NCH], f32, tag="ysb")
                    nc.scalar.copy(ysb[:], yps[:])
                    rec = recp.tile([128, NCH], f32, tag="rec")
                    nc.vector.reciprocal(rec[:], sps[:])
                    nc.vector.tensor_mul(
                        yT[:, h, tok0 + qc * NCH: tok0 + (qc + 1) * NCH],
                        ysb[:], rec[:])
                if pending is not None:
                    emit_proj(*pending)
                pending = (b, qc)
        emit_proj(*pending)

        if debug:
            nc.sync.dma_start(dbg_q.ap().rearrange("d (h t) -> d h t", h=NQH),
                              qT[:])
            nc.sync.dma_start(dbg_k.ap(), kT[:])
            nc.sync.dma_start(dbg_v.ap().rearrange("p (i d) -> p i d", i=B * KB),
                              vv[:])
            nc.sync.dma_start(dbg_y.ap().rearrange("d (h t) -> d h t", h=NQH),
                              yT[:])

    nc.compile()
    return nc


def _get_nc():
    if "nc" not in _COMPILED:
        _COMPILED["nc"] = _build_nc()
    return _COMPILED["nc"]


def _stage_inputs(x, Wq, Wkv, Wproj):
    xt = np.ascontiguousarray(
        x.reshape(TOK, C).T).astype(BF16)                       # [C, TOK]
    cos, sin = _rope_tables()
    cos = cos.astype(BF16)
    sin = sin.astype(BF16)
    kk, qq = np.meshgrid(np.arange(D), np.arange(D), indexing="ij")
    tri = (kk <= qq).astype(BF16)                               # [k, q]

    in_maps = []
    for c in range(N_CORES):
        g = c // 2
        wq = np.ascontiguousarray(
            (Wq[2 * c * D:(2 * c + 2) * D, :] * SCALE).T).astype(BF16)
        wk = np.ascontiguousarray(Wkv[g * D:(g + 1) * D, :].T).astype(BF16)
        wv = np.ascontiguousarray(
            Wkv[4 * D + g * D: 4 * D + (g + 1) * D, :].T).astype(BF16)
        wp = np.ascontiguousarray(
            Wproj[:, 2 * c * D:(2 * c + 2) * D].T).astype(BF16)
        in_maps.append({
            "xt": xt, "wq": wq, "wk": wk, "wv": wv, "wp": wp,
            "cos": cos, "sin": sin, "tri": tri,
        })
    return in_maps


def run(x, Wq, Wkv, Wproj, trace=False):
    from concourse.bass_utils import run_bass_kernel_spmd

    nc = _get_nc()
    in_maps = _stage_inputs(x, Wq, Wkv, Wproj)
    res = run_bass_kernel_spmd(nc, in_maps, core_ids=list(range(N_CORES)),
                               trace=trace)
    acc = np.zeros((TOK, C), np.float32)
    for c in range(N_CORES):
        acc += res.results[c]["out"]
    out = acc.reshape(B, T, C)
    return (out, res) if trace else (out, None)


def kernel(x, Wq, Wkv, Wproj):
    out, _ = run(np.asarray(x, np.float32), np.asarray(Wq, np.float32),
                 np.asarray(Wkv, np.float32), np.asarray(Wproj, np.float32))
    return out

